# revision 18
# baseline (speedup 1.0000x reference)
"""MultiHeadAttention (B=2, S=2048, D=1024, H=16) on 8 Trainium2 NeuronCores.

Sharding: core c -> batch b = c // 4, head group g = c % 4 (4 of 16 heads =
256 of the 1024 projection columns). Cores are fully independent (no
collectives): each core writes its partial output projection (rows x D for
its 256 ctx columns) as fp16, and the host sums the 4 partials per batch
and adds bo.

v3 schedule (evolved under trace analysis; see git-less history in
_transcript):

  - All input DMAs post up front, ordered by consumption time: wk, xk0
    (halved), wq, xq0, then xk1-3 (kT gates every scores matmul), then
    wv/xv0-3, wo, xq1-3. The DMA subsystem ramps for its first ~12us
    regardless of packet shape; the first matmul lands ~13us.
  - x tensors are host-packed [128, 4, IC, 512] so each (partition,
    512-col slice) is one contiguous 4-8KB DMA line.
  - kT stays zero-interleaved [128, 2S]: all matmul stationaries must
    be 128 rows - a stationary row-count switch costs a ~90ns PE
    pipeline bubble per matmul (measured).
  - Pre-phase is just kproj slice 0 + qproj slice 0. Everything else
    (kproj s1-3, vproj s0-3, qproj s1-3, P5 tiles of finished chunks)
    runs as fill items inside the attention combos' exp-shadow slots,
    ordered to match DMA arrival and consumption deadlines.
  - Attention: per (head, 512-q-chunk) combo, per key-block pair:
    scores -> exp -> [fills] -> attnV of the pair TWO back. The 2-pair
    lag keeps the in-order PE queue from parking behind an exp.
  - PSUM: sc/fill accs share ONE 3-slot [128,1024] pool (6 banks) +
    ov [65,512] x2 (2 banks). Three slots let consecutive fill items
    and scores rotate without waiting on each other's DVE sinks - a
    single-buffered fill acc serialized the whole front (+12us).
  - P5 uses wide [128, 1024] tiles: 4 matmuls (a single 1024-wide
    matmul output fails to compile - PSUM bank limit is 512 f32), ONE
    cast, ONE dma post (2KB packets) issued from GpSimd to keep Sync
    free. Tail P5 accs rotate through the 3 freed slots with casts
    alternating Scalar/Vector; the last normalize is split in halves.

Matmul operands are fp16 (1 cycle/row on the PE); accumulation is fp32.
fp8/DoubleRow was evaluated and rejected: quantizing any attention operand
to fp8 puts 4-9% noise directly on the output (softmax averages signal and
noise alike), far over the 2e-2 budget.
"""

import numpy as np

import concourse.bacc as bacc
import concourse.mybir as mybir
from concourse.tile import TileContext
from concourse.bass_utils import run_bass_kernel_spmd

F32 = mybir.dt.float32
F16 = mybir.dt.float16

B, S, D = 2, 2048, 1024
H, DH = 16, 64
NCORES = 8
HPG = 4            # heads per core
DG = HPG * DH      # 256 projection cols per core
IC = D // 128      # 8 contraction chunks for the projections
KC = S // 128      # 16 key blocks
VW = DH + 1        # 65 = head dim + ones column

_NC_CACHE = {}


def _build_nc():
    nc = bacc.Bacc("TRN2", target_bir_lowering=False, num_devices=NCORES)

    # x packed [128, 4, IC, 512]: per (partition, 512-col slice) the IC
    # contraction chunks are contiguous (8KB lines -> 1 DMA descriptor per
    # partition per slice; descriptor generation, not bandwidth, limits the
    # early DMA ramp)
    xq = nc.dram_tensor("xq", [128, 4, IC, 512], F16, kind="ExternalInput")
    xk = nc.dram_tensor("xk", [128, 4, IC, 512], F16, kind="ExternalInput")
    xv = nc.dram_tensor("xv", [128, 4, IC, 512], F16, kind="ExternalInput")
    # weights host-packed per-partition-contiguous: one 4KB descriptor per
    # partition on load
    wq = nc.dram_tensor("wq", [128, IC * DG], F16, kind="ExternalInput")
    wk = nc.dram_tensor("wk", [128, IC * DG], F16, kind="ExternalInput")
    wv = nc.dram_tensor("wv", [128, IC * DG], F16, kind="ExternalInput")
    wo = nc.dram_tensor("wo", [128, 2 * D], F16, kind="ExternalInput")
    bq2 = nc.dram_tensor("bq2", [2, 128], F32, kind="ExternalInput")
    bk2 = nc.dram_tensor("bk2", [2, 128], F32, kind="ExternalInput")
    bvb = nc.dram_tensor("bvb", [128, DG], F32, kind="ExternalInput")
    out = nc.dram_tensor("out", [S, D], F16, kind="ExternalOutput")

    Exp = mybir.ActivationFunctionType.Exp
    Copy = mybir.ActivationFunctionType.Copy

    with nc.allow_low_precision(reason="fp16 attention internals"), \
            TileContext(nc) as tc:
        with (
            tc.tile_pool(name="persist", bufs=1) as pers,
            tc.tile_pool(name="xin", bufs=1) as xin,
            tc.tile_pool(name="pt", bufs=4) as ptp,
            tc.tile_pool(name="small", bufs=4) as small,
            tc.tile_pool(name="outp", bufs=4) as outp,
            tc.tile_pool(name="ps2", bufs=3, space="PSUM") as ps2,
            tc.tile_pool(name="psov", bufs=2, space="PSUM") as psov,
        ):
            # ---- persistent SBUF ----
            wq_sb = pers.tile([128, IC * DG], F16, tag="wq")
            wk_sb = pers.tile([128, IC * DG], F16, tag="wk")
            wv_sb = pers.tile([128, IC * DG], F16, tag="wv")
            wo_sb = pers.tile([128, 2 * D], F16, tag="wo")
            qt_sb = [pers.tile([128, S], F16, tag=f"qt{i}", name=f"qt{i}")
                     for i in range(2)]
            ktz_sb = [pers.tile([128, 2 * S], F16, tag=f"kt{i}",
                              name=f"ktz{i}")
                      for i in range(2)]
            v_sb = pers.tile([128, KC * HPG * VW], F16, tag="v")
            ctxT_sb = [pers.tile([128, S], F16, tag=f"cx{i}", name=f"cx{i}")
                       for i in range(2)]
            bq_sb = pers.tile([128, 2], F32, tag="bq")
            bk_sb = pers.tile([128, 2], F32, tag="bk")
            bvb_sb = pers.tile([128, DG], F32, tag="bvb")

            # ---- x staging: one tile per (tensor, slice), all DMAs posted
            # up front in priority order. Arrival tracks post order at
            # ~350 GB/s aggregate once the queue is deep. ----
            xt = {}
            for nm in ("k", "q", "v"):
                for s4 in range(4):
                    xt[nm, s4] = xin.tile([128, IC * 512], F16,
                                          tag=f"x{nm}{s4}", name=f"x{nm}{s4}")

            def post_x(nm, xdr, s4, ic0=0, ic1=IC):
                nc.sync.dma_start(
                    out=xt[nm, s4].rearrange(
                        "p (k n) -> p k n", k=IC)[:, ic0:ic1, :],
                    in_=xdr[:, s4, ic0:ic1, :],
                )

            # critical path first: wk halves + xk slice 0 halves, so the
            # first matmul starts after a half-load of each. Order matches
            # the consumption schedule: pre-phase needs wk/xk0/wq/xq0/wv/xv0;
            # the remaining k/v slices are consumed as fill items inside the
            # first attention combo, interleaved k1,v1,k2,v2,k3,v3.
            nc.sync.dma_start(out=wk_sb[:, 0:IC * DG // 2],
                              in_=wk[:, 0:IC * DG // 2])
            post_x("k", xk, 0, 0, 4)
            nc.sync.dma_start(out=wk_sb[:, IC * DG // 2:],
                              in_=wk[:, IC * DG // 2:])
            post_x("k", xk, 0, 4, 8)
            nc.sync.dma_start(out=bk_sb[:], in_=bk2.rearrange("c p -> p c"))
            nc.sync.dma_start(out=wq_sb[:], in_=wq[:])
            nc.sync.dma_start(out=bq_sb[:], in_=bq2.rearrange("c p -> p c"))
            post_x("q", xq, 0)
            post_x("k", xk, 1)
            post_x("k", xk, 2)
            post_x("k", xk, 3)
            nc.sync.dma_start(out=wv_sb[:], in_=wv[:])
            nc.sync.dma_start(out=bvb_sb[:], in_=bvb[:])
            for s4 in range(4):
                post_x("v", xv, s4)
            nc.sync.dma_start(out=wo_sb[:], in_=wo[:])
            for s4 in range(1, 4):
                post_x("q", xq, s4)

            # ones columns for the softmax denominators (v is added on
            # top), and the ktz zero padding (only 64 of 128 d-rows per
            # column block are populated; zeros kill the cross-head term
            # so scores can contract the full 128 partitions - keeping all
            # stationaries 128 rows avoids the ~90ns PE bubble on a
            # stationary row-count switch)
            v4 = v_sb.rearrange("p (k h e) -> p k h e", h=HPG, e=VW)
            nc.vector.memset(v4[:, :, :, DH], 1.0)
            for z in range(2):
                nc.vector.memset(ktz_sb[z][:], 0.0)

            wq3 = wq_sb.rearrange("p (k n) -> p k n", k=IC)
            wk3 = wk_sb.rearrange("p (k n) -> p k n", k=IC)
            wv3 = wv_sb.rearrange("p (k n) -> p k n", k=IC)
            wo3 = wo_sb.rearrange("p (k n) -> p k n", k=2)

            # ---- projection helpers ----
            def ktz_sink(acc, s4, oc):
                # head hsel of this pair -> col block (2*kb + hsel)*128,
                # with only d-rows 64*hsel..+64 populated (rest stays zero)
                z4 = ktz_sb[oc].rearrange("p (kb two m) -> p kb two m",
                                          two=2, m=128)
                a3 = acc.rearrange("p (kb m) -> p kb m", m=128)
                kb0 = 4 * s4
                for hsel in range(2):
                    nc.vector.tensor_scalar_add(
                        z4[64 * hsel:64 * hsel + 64, kb0:kb0 + 4, hsel, :],
                        a3[64 * hsel:64 * hsel + 64, :, :],
                        bk_sb[64 * hsel:64 * hsel + 64, oc:oc + 1],
                    )

            def qk_proj(xtile, wsb3, dst, bias, s4, pool):
                x3 = xtile.rearrange("p (k n) -> p k n", k=IC)
                acc = pool.tile([128, 1024], F32, tag="sc",
                                name=f"qk_acc{s4}")
                for oc in range(2):
                    for ic in range(IC):
                        nc.tensor.matmul(
                            acc[:, oc * 512:(oc + 1) * 512],
                            wsb3[:, ic, oc * 128:(oc + 1) * 128],
                            x3[:, ic, :],
                            start=(ic == 0),
                            stop=(ic == IC - 1),
                        )
                for oc in range(2):
                    if dst is None:
                        ktz_sink(acc[:, oc * 512:(oc + 1) * 512], s4, oc)
                    else:
                        nc.vector.tensor_scalar_add(
                            dst[oc][:, s4 * 512:(s4 + 1) * 512],
                            acc[:, oc * 512:(oc + 1) * 512],
                            bias[:, oc:oc + 1],
                        )

            def qk_proj_half(xtile, wsb3, dst, bias, s4, oc, pool):
                x3 = xtile.rearrange("p (k n) -> p k n", k=IC)
                acc = pool.tile([128, 1024], F32, tag="sc",
                                name=f"qk_half{s4}_{oc}")
                for ic in range(IC):
                    nc.tensor.matmul(
                        acc[:, 0:512],
                        wsb3[:, ic, oc * 128:(oc + 1) * 128],
                        x3[:, ic, :],
                        start=(ic == 0),
                        stop=(ic == IC - 1),
                    )
                if dst is None:
                    ktz_sink(acc[:, 0:512], s4, oc)
                else:
                    nc.vector.tensor_scalar_add(
                        dst[oc][:, s4 * 512:(s4 + 1) * 512], acc[:, 0:512],
                        bias[:, oc:oc + 1],
                    )

            def v_proj_kc(s4, j, pool):
                kc = 4 * s4 + j
                x3 = xt["v", s4].rearrange("p (k n) -> p k n", k=IC)
                acc = pool.tile([128, 1024], F32, tag="sc",
                                name=f"v_acc{kc}")
                for ic in range(IC):
                    nc.tensor.matmul(
                        acc[:, 0:DG],
                        x3[:, ic, j * 128:(j + 1) * 128],
                        wv3[:, ic, :],
                        start=(ic == 0),
                        stop=(ic == IC - 1),
                    )
                nc.vector.tensor_add(
                    out=v4[:, kc, :, 0:DH],
                    in0=acc[:, 0:DG].rearrange("p (h e) -> p h e", e=DH),
                    in1=bvb_sb.rearrange("p (h e) -> p h e", e=DH),
                )

            # ---- P5: one [128, 1024] output tile (full D row block) ----
            def p5_tile(ib, pool, cast_engine):
                acc = pool.tile([128, 1024], F32, tag="sc",
                                name=f"p5_acc{ib}")
                for oh in range(2):
                    for cc in range(2):
                        nc.tensor.matmul(
                            acc[:, oh * 512:(oh + 1) * 512],
                            ctxT_sb[cc][:, ib * 128:(ib + 1) * 128],
                            wo3[:, cc, oh * 512:(oh + 1) * 512],
                            start=(cc == 0),
                            stop=(cc == 1),
                        )
                ot = outp.tile([128, 1024], F16, tag="ot", name=f"ot{ib}")
                if cast_engine == "scalar":
                    nc.scalar.activation(ot[:], acc[:], Copy)
                else:
                    nc.vector.tensor_copy(out=ot[:], in_=acc[:])
                nc.gpsimd.dma_start(
                    out=out[ib * 128:(ib + 1) * 128, :], in_=ot[:])

            # ---- pre-phase: slice 0 of k/q/v only; the rest becomes
            # fill work inside the first attention combo so the PE can chew
            # attention matmuls during the DMA stream-in ----
            qk_proj(xt["k", 0], wk3, None, bk_sb, 0, ps2)
            qk_proj(xt["q", 0], wq3, qt_sb, bq_sb, 0, ps2)

            # ---- fill plan: combo (chunk c, head hg) -> {kp: [closures]} ----
            def mk_v(s4, j):
                return lambda: v_proj_kc(s4, j, ps2)

            def mk_k(s4, oc):
                return lambda: qk_proj_half(xt["k", s4], wk3, None, bk_sb,
                                            s4, oc, ps2)

            def mk_q(s4, oc):
                return lambda: qk_proj_half(xt["q", s4], wq3, qt_sb, bq_sb,
                                            s4, oc, ps2)

            def mk_p5(ib):
                return lambda: p5_tile(ib, ps2, "vector")

            fill = {(c, hg): {} for c in range(4) for hg in range(4)}
            # k/v slices 1-3 land in (chunk0, head0) just ahead of their key
            # blocks (kproj-sN before scores kp=2N, vproj-sN before attnV of
            # those blocks, which trails by the 2-pair lag); DMA arrival
            # order matches.
            fill[0, 0] = {0: [mk_k(1, 0), mk_k(1, 1)],
                          1: [mk_k(2, 0), mk_k(2, 1)],
                          2: [mk_k(3, 0), mk_k(3, 1),
                              mk_v(0, 0), mk_v(0, 1)],
                          3: [mk_v(0, 2), mk_v(0, 3),
                              mk_v(1, 0), mk_v(1, 1)],
                          4: [mk_v(1, 2), mk_v(1, 3)],
                          5: [mk_v(2, 0), mk_v(2, 1)],
                          6: [mk_v(2, 2), mk_v(2, 3),
                              mk_v(3, 0), mk_v(3, 1)],
                          7: [mk_v(3, 2), mk_v(3, 3)]}
            fill[0, 1] = {3: [mk_q(1, 0)]}
            fill[0, 2] = {3: [mk_q(1, 1)]}
            for c in range(1, 4):
                for hg in range(4):
                    fill[c, hg] = {2: [mk_p5(4 * (c - 1) + hg)]}
                if c < 3:
                    fill[c, 1][5] = [mk_q(c + 1, 0)]
                    fill[c, 2][5] = [mk_q(c + 1, 1)]

            # ---- normalize: ov [65, w] -> ctxT (d rows / denominator row) ----
            def normalize(ov, oc, ofs, qlo, col0, w):
                den = small.tile([1, 512], F32, tag="den", name="den")
                rec = small.tile([1, 512], F32, tag="rec", name="rec")
                nc.vector.tensor_copy(out=den[0:1, 0:w],
                                      in_=ov[DH:VW, col0:col0 + w])
                nc.vector.reciprocal_approx_fast(rec[0:1, 0:w],
                                                 den[0:1, 0:w])
                bcs = small.tile([DH, 512], F32, tag="bcs", name="bcs")
                nc.gpsimd.partition_broadcast(bcs[:, 0:w], rec[0:1, 0:w])
                nc.vector.tensor_mul(
                    out=ctxT_sb[oc][ofs:ofs + DH, qlo + col0:qlo + col0 + w],
                    in0=ov[0:DH, col0:col0 + w],
                    in1=bcs[:, 0:w],
                )

            # ---- attention combos ----
            for c in range(4):
                qlo = c * 512
                for hg in range(4):
                    oc, ofs = hg // 2, 64 * (hg % 2)
                    items = fill[c, hg]
                    ov = psov.tile([VW, 512], F32, tag="ov", name=f"ov{c}{hg}")
                    pts = []

                    def attnv_pair(kp):
                        for i in range(2):
                            kb = 2 * kp + i
                            nc.tensor.matmul(
                                ov[:],
                                v_sb[:, (kb * HPG + hg) * VW:
                                     (kb * HPG + hg + 1) * VW],
                                pts[kp][:, i * 512:(i + 1) * 512],
                                start=(kb == 0),
                                stop=(kb == KC - 1),
                            )

                    for kp in range(KC // 2):
                        sc = ps2.tile([128, 1024], F32, tag="sc",
                                      name=f"sc{c}{hg}{kp}")
                        hsel = hg % 2
                        for i in range(2):
                            kb = 2 * kp + i
                            nc.tensor.matmul(
                                sc[:, i * 512:(i + 1) * 512],
                                ktz_sb[oc][:, (2 * kb + hsel) * 128:
                                           (2 * kb + hsel + 1) * 128],
                                qt_sb[oc][:, qlo:qlo + 512],
                                start=True,
                                stop=True,
                            )
                        pt = ptp.tile([128, 1024], F16, tag="pt",
                                      name=f"pt{c}{hg}{kp}")
                        nc.scalar.activation(pt[:], sc[:], Exp, scale=0.125)
                        pts.append(pt)
                        for fn in items.get(kp, ()):
                            fn()
                        if kp >= 2:
                            attnv_pair(kp - 2)
                    attnv_pair(KC // 2 - 2)
                    attnv_pair(KC // 2 - 1)
                    if c == 3 and hg == 3:
                        # split the very last normalize to shorten the
                        # serial tail chain
                        normalize(ov, oc, ofs, qlo, 0, 256)
                        normalize(ov, oc, ofs, qlo, 256, 256)
                    else:
                        normalize(ov, oc, ofs, qlo, 0, 512)

                if c == 3:
                    # tail P5: accs rotate through the 3 freed scores slots,
                    # casts alternating Scalar/Vector
                    for t in range(4):
                        p5_tile(12 + t, ps2,
                                "scalar" if t % 2 == 0 else "vector")

    nc.compile()
    return nc


def _get_nc():
    if "nc" not in _NC_CACHE:
        _NC_CACHE["nc"] = _build_nc()
    return _NC_CACHE["nc"]


def _prep_inputs(Q, K, V, Wq, Wk, Wv, Wo, bq, bk, bv, bo):
    f = np.float32
    h = np.float16
    Q, K, V = (np.asarray(a, f) for a in (Q, K, V))
    Wq, Wk, Wv, Wo = (np.asarray(a, f) for a in (Wq, Wk, Wv, Wo))
    bq, bk, bv, bo = (np.asarray(a, f) for a in (bq, bk, bv, bo))

    def pack_x(X):
        # [S, D] -> [128, 4, IC, 512]: per (partition, slice) the IC chunks
        # are contiguous (8KB DMA lines)
        return np.ascontiguousarray(
            X.T.reshape(IC, 128, 4, 512).transpose(1, 2, 0, 3).astype(h))

    xqs = [pack_x(Q[b]) for b in range(B)]
    xks = [pack_x(K[b]) for b in range(B)]
    xvs = [pack_x(V[b]) for b in range(B)]
    WqT, WkT, WvT, WoT = Wq.T, Wk.T, Wv.T, Wo.T

    def pack_w(wT_cols):
        # [1024, G] -> SBUF image [128 partitions, IC*G]: partition p holds
        # chunks [ic, p, :] contiguously
        G = wT_cols.shape[1]
        return np.ascontiguousarray(
            wT_cols.reshape(IC, 128, G).transpose(1, 0, 2).reshape(128, IC * G),
            dtype=h)

    def pack_wo(woT_rows):
        # [256, D] -> [128 partitions, 2*D]
        return np.ascontiguousarray(
            woT_rows.reshape(2, 128, D).transpose(1, 0, 2).reshape(128, 2 * D),
            dtype=h)

    in_maps = []
    for c in range(NCORES):
        b, g = c // 4, c % 4
        cols = slice(DG * g, DG * (g + 1))
        in_maps.append({
            "xq": xqs[b], "xk": xks[b], "xv": xvs[b],
            "wq": pack_w(WqT[:, cols]),
            "wk": pack_w(WkT[:, cols]),
            "wv": pack_w(WvT[:, cols]),
            "wo": pack_wo(WoT[cols, :]),
            "bq2": np.ascontiguousarray(bq[cols]).reshape(2, 128),
            "bk2": np.ascontiguousarray(bk[cols]).reshape(2, 128),
            "bvb": np.ascontiguousarray(np.broadcast_to(bv[cols], (128, DG))),
        })
    return in_maps, bo


def _assemble(results, bo):
    out = np.empty((B, S, D), np.float32)
    for b in range(B):
        acc = results[4 * b]["out"].astype(np.float32)
        for g in range(1, 4):
            acc += results[4 * b + g]["out"].astype(np.float32)
        out[b] = acc + bo
    return out


def kernel(**inputs):
    nc = _get_nc()
    in_maps, bo = _prep_inputs(**inputs)
    res = run_bass_kernel_spmd(nc, in_maps, core_ids=list(range(NCORES)))
    return _assemble(res.results, bo)
